# revision 6
# baseline (speedup 1.0000x reference)
"""Trainium2 Bass kernel for nn_ConditionedConvolution2D, v5 (QUAD).

Reference computation:
    A  = P @ dense_w                      # [B, 3*3*C*C_OUT] per-sample conv kernels
    Wk = A.reshape(B, 3, 3, C, C_OUT)
    Y[b] = conv2d(X[b], Wk[b])            # SAME padding, stride 1, NHWC

Strategy (pure data parallel, 4 samples per core on 8 cores):
  - Host pre-lays X as a float8_e3m4 "QUAD" im2col slab with K=128:
    planes j=0..2 are the w-shifted triple of row s-1 (dw = j), plane j=3 is
    row s at dw=0.  Each slab row s in [0,128] yields a full-width stationary
    lhsT [128=(j,ci), 128=w] in one AP.  vs the 96-partition triple this (a)
    uses all 16 SDMA engines (96-wide transfers leave the odd engines idle,
    so the extra plane ships in otherwise-dead engine time), and (b) makes
    LDWEIGHTS a full 128x128 fast-weight-load.  e3m4 (4 mantissa bits,
    ~1.4e-2 rel-l2, gate 2e-2) halves X bytes vs bf16; weights and output
    stay bf16 (fp8 weights would fail the gate).
  - Per (s, output row r): weight block beta = 2-(s-r) selects (dh, dw)
    pairs per plane: beta=2: {W00,W01,W02,W10}, beta=1: {0,W11,W12,W20},
    beta=0: {0,W21,W22,0} — each of the 9 taps counted exactly once; the
    boundary rows lose only pad-zero contributions.  The zero blocks are
    baked into the host-permuted hypernetwork weight so the 96 hypernet
    matmuls emit the conv moving operand layout directly.
  - Conv: PSUM accumulators hold 16 output rows ([128 w, 16*32]); one
    mixed-dtype matmul (stationary e3m4, moving bf16 [128, <=96]) per slab
    row writes row-chunks r = s-2, s-1, s in one shot.  Completed banks are
    cast to bf16 (alternating DVE/ACT) into a per-sample [128, 4096] staging
    tile, shipped 512KB per half-sample; host transposes back to NHWC.
"""

import os
import sys

sys.path.insert(0, "/opt/trn_rl_repo")

import numpy as np
import ml_dtypes

import concourse.bacc as bacc
import concourse.mybir as mybir
import concourse.tile as tile
from concourse.bass_utils import run_bass_kernel_spmd

B, H, W, C = 32, 128, 128, 32
P_DIM = 128
KH = KW = 3
C_OUT = 32
N_CORES = 8
BPC = B // N_CORES          # samples per core
W2 = 132                    # padded row pitch
S = H + 1                   # slab rows (s=0 carries the j=3-only edge row)
G = KH * C_OUT              # 96 weight-stream columns per sample (beta, co)
GP = 128                    # hypernet stationary width (j, ci)
RPT = 16                    # output rows per PSUM tile (one full bank)

# T[beta][j] = (dh, dw) of the tap carried by plane j in weight block beta
T = {2: {0: (0, 0), 1: (0, 1), 2: (0, 2), 3: (1, 0)},
     1: {1: (1, 1), 2: (1, 2), 3: (2, 0)},
     0: {1: (2, 1), 2: (2, 2)}}

_NC_CACHE = {}
BF16 = ml_dtypes.bfloat16
E3M4 = ml_dtypes.float8_e3m4


def _build_nc():
    f32 = mybir.dt.float32
    bf16 = mybir.dt.bfloat16
    e3m4 = mybir.dt.float8e3
    nc = bacc.Bacc("TRN2", target_bir_lowering=False, debug=False,
                   num_devices=N_CORES)
    x_quad = nc.dram_tensor("x_quad", [BPC, P_DIM, S * W2], e3m4,
                            kind="ExternalInput")
    p_t = nc.dram_tensor("p_t", [P_DIM, BPC], bf16, kind="ExternalInput")
    dw_t = nc.dram_tensor("dw_t", [P_DIM, G * GP], bf16,
                          kind="ExternalInput")
    y = nc.dram_tensor("y", [BPC, W, H * C_OUT], bf16, kind="ExternalOutput")

    with tile.TileContext(nc) as tc:
        with tc.tile_pool(name="const", bufs=1) as cpool, \
             tc.tile_pool(name="wsb", bufs=1) as wsb_pool, \
             tc.tile_pool(name="slab", bufs=3) as slab_pool, \
             tc.tile_pool(name="osb", bufs=2) as osb_pool:

            # ---- Phase 0: hypernetwork  Wk = P @ dense_w (permuted) ----
            # DMA ring split: slabs ride the SP HWDGE ring (nc.sync), the
            # hypernet weight rides the ACT HWDGE ring (nc.scalar), outputs
            # ride SWDGE (nc.gpsimd) — three independent descriptor streams
            # that the SDMA engines round-robin at packet granularity, so
            # input/weight/output traffic overlaps instead of serializing.
            p_sb = cpool.tile([P_DIM, BPC], bf16, name="p_sb", tag="p_sb")
            nc.scalar.dma_start(out=p_sb[:], in_=p_t[:])
            dwsb = cpool.tile([P_DIM, G * GP], bf16, name="dwsb", tag="dwsb")
            NSPLIT = 8
            gsz = G // NSPLIT
            for k in range(NSPLIT):
                nc.scalar.dma_start(
                    out=dwsb[:, k * gsz * GP:(k + 1) * gsz * GP],
                    in_=dw_t[:, k * gsz * GP:(k + 1) * gsz * GP])

            # w_sb[(j,ci), b*G + beta*32 + co] (bf16 stream operand)
            w_sb = wsb_pool.tile([P_DIM, BPC * G], bf16, name="w_sb",
                                 tag="w_sb")

            with tc.tile_pool(name="wps", bufs=1, space="PSUM") as wps_pool:
                wps = wps_pool.tile([P_DIM, G * BPC], f32, name="wps",
                                    tag="wps")
                for g in range(G):      # g = beta*32 + co
                    nc.tensor.matmul(
                        out=wps[:, g * BPC:(g + 1) * BPC],
                        lhsT=dwsb[:, g * GP:(g + 1) * GP],
                        rhs=p_sb[:],
                        start=True, stop=True,
                    )
                # permute (g, b) -> (b, g) while casting f32 -> bf16
                src = wps[:].rearrange("p (g b) -> p g b", b=BPC)
                dst = w_sb[:].rearrange("p (b g) -> p g b", g=G)
                nc.vector.tensor_copy(out=dst, in_=src)

            # ---- Phase 1: per-sample conv ----
            with tc.tile_pool(name="acc", bufs=3, space="PSUM") as acc_pool:
                for b in range(BPC):
                    slab = slab_pool.tile([P_DIM, S * W2], e3m4, name="slab",
                                          tag="slab")
                    HSPLIT = 65
                    nc.sync.dma_start(out=slab[:, :HSPLIT * W2],
                                      in_=x_quad[b][:, :HSPLIT * W2])
                    nc.sync.dma_start(out=slab[:, HSPLIT * W2:],
                                      in_=x_quad[b][:, HSPLIT * W2:])

                    osb = osb_pool.tile([W, H * C_OUT], bf16, name="osb",
                                        tag="osb")

                    tiles = {}      # t -> psum AP [W, RPT*C_OUT]
                    for s in range(S):
                        lhsT = slab[:, s * W2: s * W2 + W]
                        rows = [r for r in (s - 2, s - 1, s) if 0 <= r < H]
                        groups = []
                        for r in rows:
                            t = r // RPT
                            if groups and groups[-1][0] == t:
                                groups[-1][1].append(r)
                            else:
                                groups.append((t, [r]))
                        for t, rs in groups:
                            start = t not in tiles
                            if start:
                                tiles[t] = acc_pool.tile(
                                    [W, RPT * C_OUT], f32, name="acc",
                                    tag="acc")
                            r_lo = rs[0]
                            c_lo = r_lo % RPT
                            w_lo = 2 - (s - r_lo)
                            last = s == min(t * RPT + RPT + 1, H)
                            nc.tensor.matmul(
                                out=tiles[t][:, c_lo * C_OUT:
                                             (c_lo + len(rs)) * C_OUT],
                                lhsT=lhsT,
                                rhs=w_sb[:, b * G + w_lo * C_OUT:
                                         b * G + (w_lo + len(rs)) * C_OUT],
                                start=start, stop=last,
                                skip_group_check=True,
                            )
                        for t in list(tiles):
                            if s == min(t * RPT + RPT + 1, H):
                                src2 = tiles.pop(t)
                                dst2 = osb[:, t * RPT * C_OUT:
                                           (t + 1) * RPT * C_OUT]
                                if t % 2 == 0:
                                    nc.vector.tensor_copy(out=dst2,
                                                          in_=src2[:])
                                else:
                                    nc.scalar.copy(out=dst2, in_=src2[:])
                                if t == 3 or t == 7:
                                    hf = (t - 3) // 4
                                    nc.gpsimd.dma_start(
                                        out=y[b][:, hf * 4 * RPT * C_OUT:
                                                 (hf + 1) * 4 * RPT * C_OUT],
                                        in_=osb[:, hf * 4 * RPT * C_OUT:
                                                (hf + 1) * 4 * RPT * C_OUT])
    nc.finalize()
    return nc


def _get_nc():
    if "nc" not in _NC_CACHE:
        _NC_CACHE["nc"] = _build_nc()
    return _NC_CACHE["nc"]


def _prep_inputs(X, P, dense_w):
    Xb = np.ascontiguousarray(X.transpose(0, 3, 1, 2))   # [B,C,H,W] f32
    # x_quad[b, j*32+ci, s, wp]:
    #   s>=1: j<3 -> X[b, s-1, wp+j-1, ci]; j=3 -> X[b, s, wp-1, ci]
    #   s=0:  j<3 -> 0;                     j=3 -> X[b, 0, wp-1, ci]
    Xq = Xb.astype(E3M4)
    x_quad = np.zeros((B, P_DIM, S, W2), dtype=E3M4)
    for j in range(KW):
        lo = max(0, 1 - j)
        hi = W - j
        src_lo = lo + j - 1
        x_quad[:, j * C:(j + 1) * C, 1:S, lo:hi + 1] = Xq[:, :, :, src_lo:W]
    x_quad[:, 3 * C:4 * C, 0:H, 1:1 + W] = Xq
    x_quad = x_quad.reshape(B, P_DIM, S * W2)

    # hypernet weight: dwq[p, g*128 + j*32+ci] = dense_w[p, (dh,dw,ci,co)]
    # for (dh, dw) = T[g//32][j], else 0   (g = beta*32 + co)
    dwr = dense_w.reshape(P_DIM, KH, KW, C, C_OUT)
    dwq = np.zeros((P_DIM, G, 4, C), dtype=BF16)
    for beta in range(3):
        for j, (dh, dw) in T[beta].items():
            # columns g = beta*32 + co for co in [0,32)
            blk = dwr[:, dh, dw, :, :].transpose(0, 2, 1)  # [p, co, ci]
            dwq[:, beta * 32:(beta + 1) * 32, j, :] = blk.astype(BF16)
    dwq = np.ascontiguousarray(dwq.reshape(P_DIM, G * GP))

    in_maps = []
    for c in range(N_CORES):
        sl = slice(c * BPC, (c + 1) * BPC)
        in_maps.append({
            "x_quad": np.ascontiguousarray(x_quad[sl]),
            "p_t": np.ascontiguousarray(P[sl].T).astype(BF16),
            "dw_t": dwq,
        })
    return in_maps


def _run(X, P, dense_w, **spmd_kwargs):
    nc = _get_nc()
    in_maps = _prep_inputs(X, P, dense_w)
    res = run_bass_kernel_spmd(nc, in_maps, core_ids=list(range(N_CORES)),
                               **spmd_kwargs)
    outs = []
    for c in range(N_CORES):
        yv = res.results[c]["y"].astype(np.float32)
        yv = yv.reshape(BPC, W, H, C_OUT)
        outs.append(yv.transpose(0, 2, 1, 3))        # -> [b, h, w, co]
    Y = np.ascontiguousarray(np.concatenate(outs, axis=0), dtype=np.float32)
    return Y, res


def kernel(X, P, dense_w):
    Y, _ = _run(np.asarray(X), np.asarray(P), np.asarray(dense_w))
    return Y


# revision 14
# speedup vs baseline: 1.0801x; 1.0801x over previous
"""Trainium2 Bass kernel for nn_ConditionedConvolution2D, v5 (QUAD).

Reference computation:
    A  = P @ dense_w                      # [B, 3*3*C*C_OUT] per-sample conv kernels
    Wk = A.reshape(B, 3, 3, C, C_OUT)
    Y[b] = conv2d(X[b], Wk[b])            # SAME padding, stride 1, NHWC

Strategy (pure data parallel, 4 samples per core on 8 cores):
  - Host pre-lays X as a float8_e3m4 "QUAD" im2col slab with K=128:
    planes j=0..2 are the w-shifted triple of row s-1 (dw = j), plane j=3 is
    row s at dw=0.  Each slab row s in [0,128] yields a full-width stationary
    lhsT [128=(j,ci), 128=w] in one AP.  vs the 96-partition triple this (a)
    uses all 16 SDMA engines (96-wide transfers leave the odd engines idle,
    so the extra plane ships in otherwise-dead engine time), and (b) makes
    LDWEIGHTS a full 128x128 fast-weight-load.  e3m4 (4 mantissa bits,
    ~1.4e-2 rel-l2, gate 2e-2) halves X bytes vs bf16; weights and output
    stay bf16 (fp8 weights would fail the gate).
  - Per (s, output row r): weight block beta = 2-(s-r) selects (dh, dw)
    pairs per plane: beta=2: {W00,W01,W02,W10}, beta=1: {0,W11,W12,W20},
    beta=0: {0,W21,W22,0} — each of the 9 taps counted exactly once; the
    boundary rows lose only pad-zero contributions.  The zero blocks are
    baked into the host-permuted hypernetwork weight so the 96 hypernet
    matmuls emit the conv moving operand layout directly.
  - Conv: PSUM accumulators hold 16 output rows ([128 w, 16*32]); one
    mixed-dtype matmul (stationary e3m4, moving bf16 [128, <=96]) per slab
    row writes row-chunks r = s-2, s-1, s in one shot.  Completed banks are
    cast to bf16 (alternating DVE/ACT) into a per-sample [128, 4096] staging
    tile, shipped 512KB per half-sample; host transposes back to NHWC.
"""

import os
import sys

sys.path.insert(0, "/opt/trn_rl_repo")

import numpy as np
import ml_dtypes

import concourse.bacc as bacc
import concourse.mybir as mybir
import concourse.tile as tile
from concourse.bass_utils import run_bass_kernel_spmd

B, H, W, C = 32, 128, 128, 32
P_DIM = 128
KH = KW = 3
C_OUT = 32
N_CORES = 8
BPC = B // N_CORES          # samples per core
W2 = 132                    # padded row pitch
S = H + 1                   # slab rows (s=0 carries the j=3-only edge row)
G = KH * C_OUT              # 96 weight-stream columns per sample (beta, co)
RPT = 16                    # output rows per PSUM tile (one full bank)

# hypernet stationary widths: block beta involves planes j in T[beta]; the
# stationary always starts at plane j=0 (base partition 0, zero-padded
# columns where a plane carries no tap) and is trimmed at the top
BWID = {0: 96, 1: 128, 2: 128}   # beta -> stationary width (cols)
DWTOT = 32 * (96 + 128 + 128)    # 11264 total dwq columns

# T[beta][j] = (dh, dw) of the tap carried by plane j in weight block beta
T = {2: {0: (0, 0), 1: (0, 1), 2: (0, 2), 3: (1, 0)},
     1: {1: (1, 1), 2: (1, 2), 3: (2, 0)},
     0: {1: (2, 1), 2: (2, 2)}}

_NC_CACHE = {}
BF16 = ml_dtypes.bfloat16
E3M4 = ml_dtypes.float8_e3m4


def _build_nc():
    f32 = mybir.dt.float32
    bf16 = mybir.dt.bfloat16
    e3m4 = mybir.dt.float8e3
    nc = bacc.Bacc("TRN2", target_bir_lowering=False, debug=False,
                   num_devices=N_CORES)
    x_quad = nc.dram_tensor("x_quad", [BPC, P_DIM, S * W2], e3m4,
                            kind="ExternalInput")
    p_t = nc.dram_tensor("p_t", [P_DIM, BPC], bf16, kind="ExternalInput")
    dw_t = nc.dram_tensor("dw_t", [P_DIM, DWTOT], bf16,
                          kind="ExternalInput")
    y = nc.dram_tensor("y", [BPC, W, H * C_OUT], bf16, kind="ExternalOutput")

    with tile.TileContext(nc) as tc:
        with tc.tile_pool(name="const", bufs=1) as cpool, \
             tc.tile_pool(name="wsb", bufs=1) as wsb_pool, \
             tc.tile_pool(name="slab", bufs=3) as slab_pool, \
             tc.tile_pool(name="osb", bufs=4) as osb_pool:

            # ---- Phase 0: hypernetwork  Wk = P @ dense_w (permuted) ----
            # DMA ring split: slabs ride the SP HWDGE ring (nc.sync), the
            # hypernet weight rides the ACT HWDGE ring (nc.scalar), outputs
            # ride SWDGE (nc.gpsimd) — three independent descriptor streams
            # that the SDMA engines round-robin at packet granularity, so
            # input/weight/output traffic overlaps instead of serializing.
            p_sb = cpool.tile([P_DIM, BPC], bf16, name="p_sb", tag="p_sb")
            nc.scalar.dma_start(out=p_sb[:], in_=p_t[:])
            dwsb = cpool.tile([P_DIM, DWTOT], bf16, name="dwsb", tag="dwsb")
            goff = [0]
            for g in range(G):
                goff.append(goff[-1] + BWID[g // 32])
            NSPLIT = 8
            gsz = G // NSPLIT
            for k in range(NSPLIT):
                nc.scalar.dma_start(
                    out=dwsb[:, goff[k * gsz]:goff[(k + 1) * gsz]],
                    in_=dw_t[:, goff[k * gsz]:goff[(k + 1) * gsz]])

            # w_sb[(j,ci), b*G + beta*32 + co] (bf16 stream operand)
            w_sb = wsb_pool.tile([P_DIM, BPC * G], bf16, name="w_sb",
                                 tag="w_sb")

            with tc.tile_pool(name="wps", bufs=1, space="PSUM") as wps_pool:
                wps = wps_pool.tile([P_DIM, G * BPC], f32, name="wps",
                                    tag="wps")
                for g in range(G):      # g = beta*32 + co
                    wd = BWID[g // 32]
                    nc.tensor.matmul(
                        out=wps[0:wd, g * BPC:(g + 1) * BPC],
                        lhsT=dwsb[:, goff[g]:goff[g + 1]],
                        rhs=p_sb[:],
                        start=True, stop=True,
                    )
                # permute (g, b) -> (b, g) while casting f32 -> bf16
                src = wps[:].rearrange("p (g b) -> p g b", b=BPC)
                dst = w_sb[:].rearrange("p (b g) -> p g b", g=G)
                nc.vector.tensor_copy(out=dst, in_=src)
                # beta=0's j=3 block is written by no matmul (junk in PSUM)
                wv = w_sb[:].rearrange("p (b g) -> p b g", g=G)
                nc.vector.memset(wv[96:128, :, 0:32], 0.0)

            # ---- Phase 1: per-sample conv ----
            with tc.tile_pool(name="acc", bufs=3, space="PSUM") as acc_pool:
                for b in range(BPC):
                    slab = slab_pool.tile([P_DIM, S * W2], e3m4, name="slab",
                                          tag="slab")
                    HSPLIT = 65
                    nc.sync.dma_start(out=slab[:, :HSPLIT * W2],
                                      in_=x_quad[b][:, :HSPLIT * W2])
                    nc.sync.dma_start(out=slab[:, HSPLIT * W2:],
                                      in_=x_quad[b][:, HSPLIT * W2:])

                    osb = osb_pool.tile([W, H * C_OUT], bf16, name="osb",
                                        tag="osb")

                    tiles = {}      # t -> psum AP [W, RPT*C_OUT]
                    for s in range(S):
                        lhsT = slab[:, s * W2: s * W2 + W]
                        rows = [r for r in (s - 2, s - 1, s) if 0 <= r < H]
                        groups = []
                        for r in rows:
                            t = r // RPT
                            if groups and groups[-1][0] == t:
                                groups[-1][1].append(r)
                            else:
                                groups.append((t, [r]))
                        for t, rs in groups:
                            start = t not in tiles
                            if start:
                                tiles[t] = acc_pool.tile(
                                    [W, RPT * C_OUT], f32, name="acc",
                                    tag="acc")
                            r_lo = rs[0]
                            c_lo = r_lo % RPT
                            w_lo = 2 - (s - r_lo)
                            last = s == min(t * RPT + RPT + 1, H)
                            nc.tensor.matmul(
                                out=tiles[t][:, c_lo * C_OUT:
                                             (c_lo + len(rs)) * C_OUT],
                                lhsT=lhsT,
                                rhs=w_sb[:, b * G + w_lo * C_OUT:
                                         b * G + (w_lo + len(rs)) * C_OUT],
                                start=start, stop=last,
                                skip_group_check=True,
                            )
                        for t in list(tiles):
                            if s == min(t * RPT + RPT + 1, H):
                                src2 = tiles.pop(t)
                                dst2 = osb[:, t * RPT * C_OUT:
                                           (t + 1) * RPT * C_OUT]
                                if t % 2 == 0:
                                    nc.vector.tensor_copy(out=dst2,
                                                          in_=src2[:])
                                else:
                                    nc.scalar.copy(out=dst2, in_=src2[:])
                                if t == 3 or t == 7:
                                    hf = (t - 3) // 4
                                    nc.gpsimd.dma_start(
                                        out=y[b][:, hf * 4 * RPT * C_OUT:
                                                 (hf + 1) * 4 * RPT * C_OUT],
                                        in_=osb[:, hf * 4 * RPT * C_OUT:
                                                (hf + 1) * 4 * RPT * C_OUT])
    nc.finalize()
    return nc


def _get_nc():
    if "nc" not in _NC_CACHE:
        _NC_CACHE["nc"] = _build_nc()
    return _NC_CACHE["nc"]


def _prep_inputs(X, P, dense_w):
    Xb = np.ascontiguousarray(X.transpose(0, 3, 1, 2))   # [B,C,H,W] f32
    # x_quad[b, j*32+ci, s, wp]:
    #   s>=1: j<3 -> X[b, s-1, wp+j-1, ci]; j=3 -> X[b, s, wp-1, ci]
    #   s=0:  j<3 -> 0;                     j=3 -> X[b, 0, wp-1, ci]
    Xq = Xb.astype(E3M4)
    x_quad = np.zeros((B, P_DIM, S, W2), dtype=E3M4)
    for j in range(KW):
        lo = max(0, 1 - j)
        hi = W - j
        src_lo = lo + j - 1
        x_quad[:, j * C:(j + 1) * C, 1:S, lo:hi + 1] = Xq[:, :, :, src_lo:W]
    x_quad[:, 3 * C:4 * C, 0:H, 1:1 + W] = Xq
    x_quad = x_quad.reshape(B, P_DIM, S * W2)

    # hypernet weight, variable width per block:
    # dwq[p, goff[g] + j*32+ci] = dense_w[p, (dh,dw,ci,co)] for
    # (dh, dw) = T[g//32][j], else 0   (g = beta*32 + co)
    dwr = dense_w.reshape(P_DIM, KH, KW, C, C_OUT)
    dwq = np.zeros((P_DIM, DWTOT), dtype=BF16)
    goff = [0]
    for g in range(G):
        goff.append(goff[-1] + BWID[g // 32])
    for g in range(G):
        beta, co = g // 32, g % 32
        for j, (dh, dw) in T[beta].items():
            if j * 32 < BWID[beta]:
                dwq[:, goff[g] + j * 32:goff[g] + (j + 1) * 32] = \
                    dwr[:, dh, dw, :, co].astype(BF16)

    in_maps = []
    for c in range(N_CORES):
        sl = slice(c * BPC, (c + 1) * BPC)
        in_maps.append({
            "x_quad": np.ascontiguousarray(x_quad[sl]),
            "p_t": np.ascontiguousarray(P[sl].T).astype(BF16),
            "dw_t": dwq,
        })
    return in_maps


def _run(X, P, dense_w, **spmd_kwargs):
    nc = _get_nc()
    in_maps = _prep_inputs(X, P, dense_w)
    res = run_bass_kernel_spmd(nc, in_maps, core_ids=list(range(N_CORES)),
                               **spmd_kwargs)
    outs = []
    for c in range(N_CORES):
        yv = res.results[c]["y"].astype(np.float32)
        yv = yv.reshape(BPC, W, H, C_OUT)
        outs.append(yv.transpose(0, 2, 1, 3))        # -> [b, h, w, co]
    Y = np.ascontiguousarray(np.concatenate(outs, axis=0), dtype=np.float32)
    return Y, res


def kernel(X, P, dense_w):
    Y, _ = _run(np.asarray(X), np.asarray(P), np.asarray(dense_w))
    return Y
